# revision 21
# baseline (speedup 1.0000x reference)
"""Difference 3D cost volume on Trainium2 (8 NeuronCores).

out[b,c,d,h,w] = l[b,c,h,w] - r[b,c,h,w-d]  for w >= d, else 1.0
l,r: [4,32,96,312] f32  ->  out: [4,32,48,96,312] f32

Sharding: h axis (96 = 8 x 12) split across the 8 cores; per core the
partition dim is (b,c) = 128 = SBUF partitions.

The kernel is store-bandwidth-bound (it writes D=48 disparity slices of the
input size). Design, in order of impact:

1. bf16 stores: the device computes f32-in / bf16-out and the host upcasts
   to f32 during unsharding, halving store traffic (46 MB instead of 92 MB
   per core). Rounding only the subtraction RESULT keeps the relative error
   at ~2^-8 (measured 3.9e-3, gate 2e-2); rounding the inputs would be fatal
   (catastrophic cancellation where l is close to r).

2. Dual producers: one DVE makes bf16 at ~246 GB/s < the ~360 GB/s HBM
   store rate, so subs are split DVE:GpSimd = 2:1 by element count (GpSimd
   runs 2-input elemwise at ~0.42 efficiency), together ~375 GB/s.

3. Pad-once: disparities run in DECREASING order (24 groups of 2), so each
   output buffer slot sees decreasing d, and the pad region [:, :d] of a
   slot -- memset to PAD once at program start -- is never dirtied by an
   earlier (larger-d) use. Zero per-group memsets. (A "store only the valid
   region per d-slice" variant measured far worse on HW: the ~600 B
   descriptors it needs fall off the DMA descriptor-rate cliff.)

4. Ring per producer: DVE stores ride the SP HWDGE ring (nc.sync), GpSimd
   stores the ACT ring (nc.scalar). Each ring's FIFO then matches its
   producer's completion order, so a slow GpSimd store never head-of-line
   blocks ready DVE stores (worth ~15 us). Inputs load h-halved, one per
   ring, and the first 3 groups compute per h-half so the store stream
   starts as soon as possible.

Measured via repeat-slope (bench_slope.py; slope of wall time vs repeated
bodies cancels the axon tunnel's ~2.5 ms dispatch overhead): ~150 us per
body vs ~250-270 us for the f32 baseline; TimelineSim says 155 us, the
bf16 HBM-bytes floor is ~150 us.
"""

import numpy as np

import bass_rust
import concourse.bass as bass
import concourse.mybir as mybir
from concourse.bass_utils import run_bass_kernel_spmd
from concourse.tile import TileContext

# run_bass_kernel_spmd's axon trace path hard-imports antenv.axon_hooks,
# which this container doesn't ship. Provide a stub that reports "no hook"
# (bass_utils then runs untraced) so a BASS_TRACE=1 environment doesn't
# crash the kernel. A real antenv, if present, wins.
try:
    import antenv.axon_hooks  # noqa: F401
except ImportError:
    import sys as _sys
    import types as _types

    _m = _types.ModuleType("antenv.axon_hooks")
    _m.get_axon_ntff_profile_hook = lambda: None
    _sys.modules["antenv.axon_hooks"] = _m

B, C, H, W = 4, 32, 96, 312
D = 48
PAD = 1.0
NCORES = 8
HL = H // NCORES          # h rows per core
P = B * C                 # 128 = SBUF partitions

F32 = mybir.dt.float32
BF16 = mybir.dt.bfloat16

# Disparity groups, decreasing d. Group g covers d in
# [d_hi-g_size+1 .. d_hi]; slot j within the group holds dj = d_lo + j.
SIZES = [2] * 24                               # sum = 48
# 16 of 48 disparities; no GpSimd group near the tail -- the widest (small-d)
# groups land on the faster DVE so the pipeline tail stays short
GPSIMD_GROUPS = frozenset({1, 4, 7, 10, 13, 16, 19, 21})
DVE_BUFS = 7
GP_BUFS = 4


def _legalize_single_wait(nc):
    """Split multi-wait sync_info into single-wait NoOps.

    The walrus build in this container rejects any instruction carrying more
    than one sync-wait command ("Too many sync wait commands"), which rules
    out Tile's stock output (multi-wait TensorTensor / tail Drain). Hoisting
    every wait of a multi-wait instruction onto its own NoOp on the same
    engine is semantically identical: the sequencer blocks on each NoOp in
    order before issuing the original instruction.
    """
    n = 0
    for fn in nc.m.functions:
        for blk in fn.blocks:
            out = []
            for ins in blk.instructions:
                si = ins.sync_info
                waits = list(si.on_wait) if si is not None and si.on_wait else []
                if len(waits) > 1:
                    for w in waits:
                        n += 1
                        nop = bass_rust.InstNoOp(name=f"splitw-{n}", engine=ins.engine)
                        nop.sync_info = mybir.SyncInfo(on_wait=[w], on_update=[])
                        out.append(nop)
                    ins.sync_info = mybir.SyncInfo(
                        on_wait=[], on_update=list(si.on_update or [])
                    )
                out.append(ins)
            blk.instructions = out
    return n


def _build_nc(
    repeat=1,
    sizes=None,
    gp_groups=None,
    dve_bufs=None,
    gp_bufs=None,
    gp_memsets=True,
    head_split=3,
    pad_host=False,
    ascending=None,
):
    if ascending is None:
        ascending = bool(pad_host)
    # the pad-once trick relies on decreasing d per buffer slot
    assert not (ascending and not pad_host)
    sizes = SIZES if sizes is None else sizes
    gp_groups = GPSIMD_GROUPS if gp_groups is None else gp_groups
    dve_bufs = DVE_BUFS if dve_bufs is None else dve_bufs
    gp_bufs = GP_BUFS if gp_bufs is None else gp_bufs
    assert sum(sizes) == D
    HH = HL // 2
    nc = bass.Bass()
    l = nc.dram_tensor("l", [P, HL, W], F32, kind="ExternalInput")
    r = nc.dram_tensor("r", [P, HL, W], F32, kind="ExternalInput")
    o = nc.dram_tensor("o", [P, D, HL, W], BF16, kind="ExternalOutput")
    gmax = max(sizes)
    with TileContext(nc) as tc:
        with tc.tile_pool(name="bufs", bufs=1) as pool:
            lt = pool.tile([P, HL, W], F32, tag="l")
            rt = pool.tile([P, HL, W], F32, tag="r")
            obufs = []
            for i in range(dve_bufs + gp_bufs):
                ob = pool.tile([P, gmax, HL, W], BF16, tag=f"ob{i}", name=f"ob{i}")
                obufs.append(ob)
            for rep in range(repeat):
                # inputs split in h-halves, one per HWDGE ring, so the first
                # (h-split) subs can start as soon as the first halves land
                nc.scalar.dma_start(out=lt[:, :HH], in_=l[:, :HH])
                nc.sync.dma_start(out=rt[:, :HH], in_=r[:, :HH])
                nc.scalar.dma_start(out=lt[:, HH:], in_=l[:, HH:])
                nc.sync.dma_start(out=rt[:, HH:], in_=r[:, HH:])

                if not pad_host:
                    # One-time pad fill, narrowed to each buffer's first-use
                    # d range (covers every slot's largest-d first use; later
                    # uses only need pad regions that earlier larger-d subs
                    # left untouched). First-use d_hi per buffer:
                    first_hi = {}
                    dd_hi, dv, gp = D - 1, 0, 0
                    for g, size in enumerate(sizes):
                        if g in gp_groups:
                            b = dve_bufs + gp % gp_bufs
                            gp += 1
                        else:
                            b = dv % dve_bufs
                            dv += 1
                        if b not in first_hi:
                            first_hi[b] = dd_hi if not ascending else D - 1
                        dd_hi -= size
                    for i, ob in enumerate(obufs):
                        eng = nc.gpsimd if gp_memsets or i >= dve_bufs else nc.vector
                        eng.memset(ob[:, :, :, : first_hi[i] + 1], PAD)

                d_hi = D - 1
                d_asc = 0
                dve_i = 0
                gp_i = 0
                n_store = 0
                for g, size in enumerate(sizes):
                    if g in gp_groups:
                        eng = nc.gpsimd
                        ob = obufs[dve_bufs + gp_i % gp_bufs]
                        gp_i += 1
                    else:
                        eng = nc.vector
                        ob = obufs[dve_i % dve_bufs]
                        dve_i += 1
                    # pad_host mode: no pad invariant to maintain, order free
                    # (non-pad_host REQUIRES descending for the pad trick)
                    d_lo = d_asc if ascending else d_hi - size + 1
                    for j in range(size):
                        dj = d_lo + j
                        hs = [(0, HH), (HH, HL)] if g < head_split else [(0, HL)]
                        for h0, h1 in hs:
                            eng.tensor_sub(
                                out=ob[:, j, h0:h1, dj:],
                                in0=lt[:, h0:h1, dj:],
                                in1=rt[:, h0:h1, : W - dj],
                            )
                    if pad_host == "slice":
                        # store only the valid region of each d-slice; the
                        # host fills the constant pad during unsharding
                        for j in range(size):
                            dj = d_lo + j
                            ring = nc.sync if n_store % 2 == 0 else nc.scalar
                            n_store += 1
                            ring.dma_start(
                                out=o[:, dj, :, dj:], in_=ob[:, j, :, dj:]
                            )
                    elif pad_host:
                        # fat contiguous store incl. garbage pad columns; the
                        # host overwrites the pad during unsharding
                        ring = nc.sync if g % 2 == 0 else nc.scalar
                        ring.dma_start(
                            out=o[:, d_lo : d_lo + size], in_=ob[:, :size]
                        )
                    else:
                        # ring dedicated per producing engine: a ring's FIFO
                        # then matches its producer's completion order, so a
                        # slow GpSimd store never head-of-line-blocks ready
                        # DVE stores (and vice versa)
                        ring = nc.scalar if g in gp_groups else nc.sync
                        ring.dma_start(
                            out=o[:, d_lo : d_hi + 1], in_=ob[:, :size]
                        )
                    d_hi -= size
                    d_asc += size
                assert d_hi == -1
    _legalize_single_wait(nc)
    return nc


_nc = None


def _in_maps(l_fmap, r_fmap):
    l = np.ascontiguousarray(l_fmap, dtype=np.float32)
    r = np.ascontiguousarray(r_fmap, dtype=np.float32)
    assert l.shape == (B, C, H, W), l.shape
    assert r.shape == (B, C, H, W), r.shape
    maps = []
    for k in range(NCORES):
        sl = slice(k * HL, (k + 1) * HL)
        maps.append(
            {
                "l": np.ascontiguousarray(l[:, :, sl, :]).reshape(P, HL, W),
                "r": np.ascontiguousarray(r[:, :, sl, :]).reshape(P, HL, W),
            }
        )
    return maps


def _gather(results):
    shards = [
        np.asarray(results[k]["o"]).astype(np.float32).reshape(B, C, D, HL, W)
        for k in range(NCORES)
    ]
    return np.concatenate(shards, axis=3)


def run(l_fmap, r_fmap, **spmd_kwargs):
    global _nc
    if _nc is None:
        _nc = _build_nc()
    res = run_bass_kernel_spmd(
        _nc, _in_maps(l_fmap, r_fmap), core_ids=list(range(NCORES)), **spmd_kwargs
    )
    return _gather(res.results), res


def kernel(l_fmap, r_fmap):
    out, _ = run(l_fmap, r_fmap)
    return out
